# revision 3
# baseline (speedup 1.0000x reference)
"""CrossModalAttention Trainium2 kernel.

Math: with seq_len=1 on both query and key/value sides, softmax over the
single key is exactly 1.0, so MHA(q_in, kv_in) == (kv_in @ Wv.T + bv) @ out_w.T + out_b.
Folding the two projections on the host (in float64):
    W = out_w @ Wv          c = bv @ out_w.T + out_b
gives   out_m = LayerNorm(kv @ W.T + c + residual) * g + b.

Device work per modality: one [B,1024]x[1024,1024] matmul + residual add +
LayerNorm.  Sharding: pure data parallel over the batch dim, 8 cores.
"""

import numpy as np

P = 128          # partitions
D = 1024         # hidden dim
KO = D // P      # 8 contraction chunks
N_CORES = 8
B_FULL = 16384
B_CORE = B_FULL // N_CORES   # 2048
RT = B_CORE // P             # 16 row tiles per core
LN_EPS = 1e-5

_PROGRAM_CACHE = {}


def _build_program(flags):
    """Build the Bass program. flags = (add_bias1, add_bias2, gb1, gb2)."""
    import concourse.bass as bass
    import concourse.bacc as bacc
    import concourse.tile as tile
    from concourse import mybir
    from concourse.masks import make_identity
    from concourse._compat import get_trn_type

    add_bias1, add_bias2, gb1, gb2 = flags
    f32 = mybir.dt.float32
    f32r = mybir.dt.float32r

    nc = bacc.Bacc(get_trn_type() or "TRN2", target_bir_lowering=False,
                   debug=False, num_devices=N_CORES)

    img = nc.dram_tensor("img", (B_CORE, D), f32, kind="ExternalInput").ap()
    txt = nc.dram_tensor("txt", (B_CORE, D), f32, kind="ExternalInput").ap()
    # weights pre-arranged on host: w[p, j, n] = W[n, j*128+p]  (i.e. W.T chunked)
    w1t = nc.dram_tensor("w1t", (P, KO, D), f32, kind="ExternalInput").ap()
    w2t = nc.dram_tensor("w2t", (P, KO, D), f32, kind="ExternalInput").ap()
    aux_names = []
    if add_bias1:
        aux_names.append("c1")
    if add_bias2:
        aux_names.append("c2")
    if gb1:
        aux_names += ["g1", "b1"]
    if gb2:
        aux_names += ["g2", "b2"]
    aux = {n: nc.dram_tensor(n, (1, D), f32, kind="ExternalInput").ap()
           for n in aux_names}
    out1 = nc.dram_tensor("out1", (B_CORE, D), f32, kind="ExternalOutput").ap()
    out2 = nc.dram_tensor("out2", (B_CORE, D), f32, kind="ExternalOutput").ap()

    with tile.TileContext(nc) as tc:
        import contextlib
        with contextlib.ExitStack() as ctx:
            const = ctx.enter_context(tc.tile_pool(name="const", bufs=1))
            feat = ctx.enter_context(tc.tile_pool(name="feat", bufs=3))
            kvtp = ctx.enter_context(tc.tile_pool(name="kvtp", bufs=3))
            sp = ctx.enter_context(tc.tile_pool(name="sp", bufs=3))
            op = ctx.enter_context(tc.tile_pool(name="op", bufs=3))
            stat = ctx.enter_context(tc.tile_pool(name="stat", bufs=6))
            psum_t = ctx.enter_context(
                tc.tile_pool(name="psum_t", bufs=4, space="PSUM"))
            psum_o = ctx.enter_context(
                tc.tile_pool(name="psum_o", bufs=4, space="PSUM"))

            ident = const.tile([P, P], f32, tag="ident")
            make_identity(nc, ident)
            eps = const.tile([P, 1], f32, tag="eps")
            nc.vector.memset(eps, LN_EPS)

            w1_sb = const.tile([P, KO, D], f32, tag="w1")
            nc.sync.dma_start(w1_sb, w1t)
            w2_sb = const.tile([P, KO, D], f32, tag="w2")
            nc.sync.dma_start(w2_sb, w2t)

            # broadcast-replicated aux rows ([1, D] dram -> [P, D] sbuf)
            aux_sb = {}
            for n, ap in aux.items():
                t = const.tile([P, D], f32, tag=n)
                bcast = bass.AP(tensor=ap.tensor, offset=ap.offset,
                                ap=[[0, P], ap.ap[1]])
                nc.sync.dma_start(t, bcast)
                aux_sb[n] = t

            for rt in range(RT):
                rows = slice(rt * P, (rt + 1) * P)
                img_sb = feat.tile([P, D], f32, tag="img")
                nc.sync.dma_start(img_sb, img[rows, :])
                txt_sb = feat.tile([P, D], f32, tag="txt")
                nc.sync.dma_start(txt_sb, txt[rows, :])

                # modality 1: kv=txt, residual=img -> out1
                # modality 2: kv=img, residual=txt -> out2
                for mod, kv_sb, res_sb, w_sb, out_d, biask, gbk in (
                    (1, txt_sb, img_sb, w1_sb, out1, add_bias1, gb1),
                    (2, img_sb, txt_sb, w2_sb, out2, add_bias2, gb2),
                ):
                    kvT = kvtp.tile([P, D], f32, tag="kvT")
                    for half in range(2):
                        ps_t = psum_t.tile([P, 512], f32, tag="ps_t")
                        for jj in range(4):
                            j = half * 4 + jj
                            nc.tensor.transpose(
                                ps_t[:, jj * P:(jj + 1) * P],
                                kv_sb[:, j * P:(j + 1) * P],
                                ident)
                        nc.any.tensor_copy(
                            out=kvT[:, half * 512:(half + 1) * 512],
                            in_=ps_t)

                    s_sb = sp.tile([P, D], f32, tag="s")
                    for nh in range(2):
                        ncol = slice(nh * 512, (nh + 1) * 512)
                        ps_o = psum_o.tile([P, 512], f32, tag="ps_o")
                        for j in range(KO):
                            nc.tensor.matmul(
                                ps_o,
                                kvT[:, j * P:(j + 1) * P],
                                w_sb[:, j, ncol],
                                start=(j == 0), stop=(j == KO - 1))
                        # s = matmul + residual
                        nc.vector.tensor_add(
                            out=s_sb[:, ncol], in0=ps_o, in1=res_sb[:, ncol])
                        if biask:
                            nc.vector.tensor_add(
                                out=s_sb[:, ncol], in0=s_sb[:, ncol],
                                in1=aux_sb[f"c{mod}"][:, ncol])

                    # layernorm over free axis
                    stats = stat.tile([P, 2, 6], f32, tag="stats")
                    nc.vector.bn_stats(stats[:, 0, :], s_sb[:, 0:512])
                    nc.vector.bn_stats(stats[:, 1, :], s_sb[:, 512:1024])
                    mv = stat.tile([P, 2], f32, tag="mv")
                    nc.vector.bn_aggr(mv, stats)
                    # mv[:,1] = 1/sqrt(var + eps)
                    nc.scalar.activation(
                        out=mv[:, 1:2], in_=mv[:, 1:2],
                        func=mybir.ActivationFunctionType.Sqrt,
                        bias=eps, scale=1.0)
                    nc.vector.reciprocal(mv[:, 1:2], mv[:, 1:2])

                    o_sb = op.tile([P, D], f32, tag="o")
                    nc.vector.tensor_scalar(
                        out=o_sb, in0=s_sb,
                        scalar1=mv[:, 0:1], scalar2=mv[:, 1:2],
                        op0=mybir.AluOpType.subtract,
                        op1=mybir.AluOpType.mult)
                    if gbk:
                        nc.vector.tensor_mul(
                            out=o_sb, in0=o_sb, in1=aux_sb[f"g{mod}"])
                        nc.vector.tensor_add(
                            out=o_sb, in0=o_sb, in1=aux_sb[f"b{mod}"])
                    nc.sync.dma_start(out_d[rows, :], o_sb)

    nc.compile()
    return nc


def _fold(in_w, in_b, out_w, out_b):
    Dv = out_w.shape[0]
    Wv = in_w[2 * Dv:3 * Dv, :].astype(np.float64)
    bv = in_b[2 * Dv:3 * Dv].astype(np.float64)
    W = (out_w.astype(np.float64) @ Wv).astype(np.float32)
    c = (bv @ out_w.astype(np.float64).T + out_b.astype(np.float64)
         ).astype(np.float32)
    # rearrange W.T [k, n] -> [p, j, n] with k = j*128 + p
    wt = np.ascontiguousarray(
        W.T.reshape(KO, P, D).transpose(1, 0, 2)).astype(np.float32)
    return wt, c


def kernel(image_features, text_features,
           in_w1, in_b1, out_w1, out_b1,
           in_w2, in_b2, out_w2, out_b2,
           ln1_g, ln1_b, ln2_g, ln2_b):
    from concourse import bass_utils

    image_features = np.ascontiguousarray(image_features, dtype=np.float32)
    text_features = np.ascontiguousarray(text_features, dtype=np.float32)

    w1t, c1 = _fold(np.asarray(in_w1), np.asarray(in_b1),
                    np.asarray(out_w1), np.asarray(out_b1))
    w2t, c2 = _fold(np.asarray(in_w2), np.asarray(in_b2),
                    np.asarray(out_w2), np.asarray(out_b2))

    flags = (bool(np.any(c1)), bool(np.any(c2)),
             bool(np.any(np.asarray(ln1_g) != 1) or np.any(np.asarray(ln1_b))),
             bool(np.any(np.asarray(ln2_g) != 1) or np.any(np.asarray(ln2_b))))

    if flags not in _PROGRAM_CACHE:
        _PROGRAM_CACHE[flags] = _build_program(flags)
    nc = _PROGRAM_CACHE[flags]

    in_maps = []
    for c in range(N_CORES):
        rows = slice(c * B_CORE, (c + 1) * B_CORE)
        m = {
            "img": np.ascontiguousarray(image_features[rows]),
            "txt": np.ascontiguousarray(text_features[rows]),
            "w1t": w1t,
            "w2t": w2t,
        }
        if flags[0]:
            m["c1"] = c1.reshape(1, D)
        if flags[1]:
            m["c2"] = c2.reshape(1, D)
        if flags[2]:
            m["g1"] = np.asarray(ln1_g, np.float32).reshape(1, D)
            m["b1"] = np.asarray(ln1_b, np.float32).reshape(1, D)
        if flags[3]:
            m["g2"] = np.asarray(ln2_g, np.float32).reshape(1, D)
            m["b2"] = np.asarray(ln2_b, np.float32).reshape(1, D)
        in_maps.append(m)

    global _LAST_IN_MAPS
    _LAST_IN_MAPS = in_maps
    res = bass_utils.run_bass_kernel_spmd(nc, in_maps, list(range(N_CORES)))
    attended_image = np.concatenate(
        [res.results[c]["out1"] for c in range(N_CORES)], axis=0)
    attended_text = np.concatenate(
        [res.results[c]["out2"] for c in range(N_CORES)], axis=0)
    return attended_image, attended_text


# revision 6
# speedup vs baseline: 2.6302x; 2.6302x over previous
"""CrossModalAttention Trainium2 kernel.

Math: with seq_len=1 on both query and key/value sides, softmax over the
single key is exactly 1.0, so MHA(q_in, kv_in) == (kv_in @ Wv.T + bv) @ out_w.T + out_b.
Folding the two projections on the host (in float64):
    W = out_w @ Wv          c = bv @ out_w.T + out_b
gives   out_m = LayerNorm(kv @ W.T + c + residual) * g + b.

Device work per modality: one [B,1024]x[1024,1024] matmul + residual add +
LayerNorm.  Sharding: pure data parallel over the batch dim, 8 cores.
"""

import numpy as np

P = 128          # partitions
D = 1024         # hidden dim
KO = D // P      # 8 contraction chunks
N_CORES = 8
B_FULL = 16384
B_CORE = B_FULL // N_CORES   # 2048
RT = B_CORE // P             # 16 row tiles per core
LN_EPS = 1e-5

_PROGRAM_CACHE = {}


def _build_program(flags):
    """Build the Bass program. flags = (add_bias1, add_bias2, gb1, gb2)."""
    import concourse.bass as bass
    import concourse.bacc as bacc
    import concourse.tile as tile
    from concourse import mybir
    from concourse.masks import make_identity
    from concourse._compat import get_trn_type

    add_bias1, add_bias2, gb1, gb2 = flags
    f32 = mybir.dt.float32
    f32r = mybir.dt.float32r

    nc = bacc.Bacc(get_trn_type() or "TRN2", target_bir_lowering=False,
                   debug=False, num_devices=N_CORES)

    img = nc.dram_tensor("img", (B_CORE, D), f32, kind="ExternalInput").ap()
    txt = nc.dram_tensor("txt", (B_CORE, D), f32, kind="ExternalInput").ap()
    # weights pre-arranged on host: w[p, j, n] = W[n, j*128+p]  (i.e. W.T chunked)
    w1t = nc.dram_tensor("w1t", (P, KO, D), f32, kind="ExternalInput").ap()
    w2t = nc.dram_tensor("w2t", (P, KO, D), f32, kind="ExternalInput").ap()
    aux_names = []
    if add_bias1:
        aux_names.append("c1")
    if add_bias2:
        aux_names.append("c2")
    if gb1:
        aux_names += ["g1", "b1"]
    if gb2:
        aux_names += ["g2", "b2"]
    aux = {n: nc.dram_tensor(n, (1, D), f32, kind="ExternalInput").ap()
           for n in aux_names}
    out1 = nc.dram_tensor("out1", (B_CORE, D), f32, kind="ExternalOutput").ap()
    out2 = nc.dram_tensor("out2", (B_CORE, D), f32, kind="ExternalOutput").ap()

    with tile.TileContext(nc) as tc:
        import contextlib
        with contextlib.ExitStack() as ctx:
            const = ctx.enter_context(tc.tile_pool(name="const", bufs=1))
            feat = ctx.enter_context(tc.tile_pool(name="feat", bufs=3))
            kvtp = ctx.enter_context(tc.tile_pool(name="kvtp", bufs=3))
            sp = ctx.enter_context(tc.tile_pool(name="sp", bufs=3))
            op = ctx.enter_context(tc.tile_pool(name="op", bufs=3))
            stat = ctx.enter_context(tc.tile_pool(name="stat", bufs=6))
            psum_t = ctx.enter_context(
                tc.tile_pool(name="psum_t", bufs=4, space="PSUM"))
            psum_o = ctx.enter_context(
                tc.tile_pool(name="psum_o", bufs=4, space="PSUM"))

            ident = const.tile([P, P], f32, tag="ident")
            make_identity(nc, ident)
            eps = const.tile([P, 1], f32, tag="eps")
            nc.vector.memset(eps, LN_EPS)

            # weights: DMA fp32, one-time rounding cast to float32r for the
            # fast single-pass PE matmul mode
            w1_sb = const.tile([P, KO, D], f32r, tag="w1")
            w2_sb = const.tile([P, KO, D], f32r, tag="w2")
            wstg = ctx.enter_context(tc.tile_pool(name="wstg", bufs=1))
            for w_sb, w_dram in ((w1_sb, w1t), (w2_sb, w2t)):
                stage = wstg.tile([P, KO, D], f32, tag="wstage")
                nc.sync.dma_start(stage, w_dram)
                for j in range(KO):
                    nc.any.tensor_copy(out=w_sb[:, j, :], in_=stage[:, j, :])

            # broadcast-replicated aux rows ([1, D] dram -> [P, D] sbuf)
            aux_sb = {}
            for n, ap in aux.items():
                t = const.tile([P, D], f32, tag=n)
                bcast = bass.AP(tensor=ap.tensor, offset=ap.offset,
                                ap=[[0, P], ap.ap[1]])
                nc.sync.dma_start(t, bcast)
                aux_sb[n] = t

            for rt in range(RT):
                rows = slice(rt * P, (rt + 1) * P)
                img_sb = feat.tile([P, D], f32, tag="img")
                nc.sync.dma_start(img_sb, img[rows, :])
                txt_sb = feat.tile([P, D], f32, tag="txt")
                nc.sync.dma_start(txt_sb, txt[rows, :])

                # modality 1: kv=txt, residual=img -> out1
                # modality 2: kv=img, residual=txt -> out2
                for mod, kv_sb, res_sb, w_sb, out_d, biask, gbk in (
                    (1, txt_sb, img_sb, w1_sb, out1, add_bias1, gb1),
                    (2, img_sb, txt_sb, w2_sb, out2, add_bias2, gb2),
                ):
                    kvT = kvtp.tile([P, D], f32r, tag="kvT")
                    for half in range(2):
                        ps_t = psum_t.tile([P, 512], f32, tag="ps_t")
                        for jj in range(4):
                            j = half * 4 + jj
                            nc.tensor.transpose(
                                ps_t[:, jj * P:(jj + 1) * P],
                                kv_sb[:, j * P:(j + 1) * P],
                                ident)
                        nc.any.tensor_copy(
                            out=kvT[:, half * 512:(half + 1) * 512],
                            in_=ps_t)

                    s_sb = sp.tile([P, D], f32, tag="s")
                    for nh in range(2):
                        ncol = slice(nh * 512, (nh + 1) * 512)
                        ps_o = psum_o.tile([P, 512], f32, tag="ps_o")
                        for j in range(KO):
                            nc.tensor.matmul(
                                ps_o,
                                kvT[:, j * P:(j + 1) * P],
                                w_sb[:, j, ncol],
                                start=(j == 0), stop=(j == KO - 1))
                        # s = matmul + residual
                        nc.vector.tensor_add(
                            out=s_sb[:, ncol], in0=ps_o, in1=res_sb[:, ncol])
                        if biask:
                            nc.vector.tensor_add(
                                out=s_sb[:, ncol], in0=s_sb[:, ncol],
                                in1=aux_sb[f"c{mod}"][:, ncol])

                    # layernorm over free axis
                    stats = stat.tile([P, 2, 6], f32, tag="stats")
                    nc.vector.bn_stats(stats[:, 0, :], s_sb[:, 0:512])
                    nc.vector.bn_stats(stats[:, 1, :], s_sb[:, 512:1024])
                    mv = stat.tile([P, 2], f32, tag="mv")
                    nc.vector.bn_aggr(mv, stats)
                    # mv[:,1] = 1/sqrt(var + eps)
                    nc.scalar.activation(
                        out=mv[:, 1:2], in_=mv[:, 1:2],
                        func=mybir.ActivationFunctionType.Sqrt,
                        bias=eps, scale=1.0)
                    nc.vector.reciprocal(mv[:, 1:2], mv[:, 1:2])

                    o_sb = op.tile([P, D], f32, tag="o")
                    nc.vector.tensor_scalar(
                        out=o_sb, in0=s_sb,
                        scalar1=mv[:, 0:1], scalar2=mv[:, 1:2],
                        op0=mybir.AluOpType.subtract,
                        op1=mybir.AluOpType.mult)
                    if gbk:
                        nc.vector.tensor_mul(
                            out=o_sb, in0=o_sb, in1=aux_sb[f"g{mod}"])
                        nc.vector.tensor_add(
                            out=o_sb, in0=o_sb, in1=aux_sb[f"b{mod}"])
                    nc.sync.dma_start(out_d[rows, :], o_sb)

    nc.compile()
    return nc


def _fold(in_w, in_b, out_w, out_b):
    Dv = out_w.shape[0]
    Wv = in_w[2 * Dv:3 * Dv, :].astype(np.float64)
    bv = in_b[2 * Dv:3 * Dv].astype(np.float64)
    W = (out_w.astype(np.float64) @ Wv).astype(np.float32)
    c = (bv @ out_w.astype(np.float64).T + out_b.astype(np.float64)
         ).astype(np.float32)
    # rearrange W.T [k, n] -> [p, j, n] with k = j*128 + p
    wt = np.ascontiguousarray(
        W.T.reshape(KO, P, D).transpose(1, 0, 2)).astype(np.float32)
    return wt, c


def kernel(image_features, text_features,
           in_w1, in_b1, out_w1, out_b1,
           in_w2, in_b2, out_w2, out_b2,
           ln1_g, ln1_b, ln2_g, ln2_b):
    from concourse import bass_utils

    image_features = np.ascontiguousarray(image_features, dtype=np.float32)
    text_features = np.ascontiguousarray(text_features, dtype=np.float32)

    w1t, c1 = _fold(np.asarray(in_w1), np.asarray(in_b1),
                    np.asarray(out_w1), np.asarray(out_b1))
    w2t, c2 = _fold(np.asarray(in_w2), np.asarray(in_b2),
                    np.asarray(out_w2), np.asarray(out_b2))

    flags = (bool(np.any(c1)), bool(np.any(c2)),
             bool(np.any(np.asarray(ln1_g) != 1) or np.any(np.asarray(ln1_b))),
             bool(np.any(np.asarray(ln2_g) != 1) or np.any(np.asarray(ln2_b))))

    if flags not in _PROGRAM_CACHE:
        _PROGRAM_CACHE[flags] = _build_program(flags)
    nc = _PROGRAM_CACHE[flags]

    in_maps = []
    for c in range(N_CORES):
        rows = slice(c * B_CORE, (c + 1) * B_CORE)
        m = {
            "img": np.ascontiguousarray(image_features[rows]),
            "txt": np.ascontiguousarray(text_features[rows]),
            "w1t": w1t,
            "w2t": w2t,
        }
        if flags[0]:
            m["c1"] = c1.reshape(1, D)
        if flags[1]:
            m["c2"] = c2.reshape(1, D)
        if flags[2]:
            m["g1"] = np.asarray(ln1_g, np.float32).reshape(1, D)
            m["b1"] = np.asarray(ln1_b, np.float32).reshape(1, D)
        if flags[3]:
            m["g2"] = np.asarray(ln2_g, np.float32).reshape(1, D)
            m["b2"] = np.asarray(ln2_b, np.float32).reshape(1, D)
        in_maps.append(m)

    global _LAST_IN_MAPS
    _LAST_IN_MAPS = in_maps
    res = bass_utils.run_bass_kernel_spmd(nc, in_maps, list(range(N_CORES)))
    attended_image = np.concatenate(
        [res.results[c]["out1"] for c in range(N_CORES)], axis=0)
    attended_text = np.concatenate(
        [res.results[c]["out2"] for c in range(N_CORES)], axis=0)
    return attended_image, attended_text


# revision 14
# speedup vs baseline: 2.6571x; 1.0102x over previous
"""CrossModalAttention Trainium2 kernel.

Math: with seq_len=1 on both query and key/value sides, softmax over the
single key is exactly 1.0, so MHA(q_in, kv_in) == (kv_in @ Wv.T + bv) @ out_w.T + out_b.
Folding the two projections on the host (in float64):
    W = out_w @ Wv          c = bv @ out_w.T + out_b
gives   out_m = LayerNorm(kv @ W.T + c + residual) * g + b.

Device work per modality: one [B,1024]x[1024,1024] matmul + residual add +
LayerNorm.  Sharding: pure data parallel over the batch dim, 8 cores.
"""

import numpy as np

P = 128          # partitions
D = 1024         # hidden dim
KO = D // P      # 8 contraction chunks
N_CORES = 8
B_FULL = 16384
B_CORE = B_FULL // N_CORES   # 2048
RT = B_CORE // P             # 16 row tiles per core
LN_EPS = 1e-5

_PROGRAM_CACHE = {}


def _build_program(flags):
    """Build the Bass program. flags = (add_bias1, add_bias2, gb1, gb2)."""
    import concourse.bass as bass
    import concourse.bacc as bacc
    import concourse.tile as tile
    from concourse import mybir
    from concourse.masks import make_identity
    from concourse._compat import get_trn_type

    add_bias1, add_bias2, gb1, gb2 = flags
    f32 = mybir.dt.float32
    f32r = mybir.dt.float32r

    nc = bacc.Bacc(get_trn_type() or "TRN2", target_bir_lowering=False,
                   debug=False, num_devices=N_CORES)

    img = nc.dram_tensor("img", (B_CORE, D), f32, kind="ExternalInput").ap()
    txt = nc.dram_tensor("txt", (B_CORE, D), f32, kind="ExternalInput").ap()
    # weights pre-arranged on host: w[p, j, n] = W[n, j*128+p]  (i.e. W.T chunked)
    w1t = nc.dram_tensor("w1t", (P, KO, D), f32r, kind="ExternalInput").ap()
    w2t = nc.dram_tensor("w2t", (P, KO, D), f32r, kind="ExternalInput").ap()
    aux_names = []
    if add_bias1:
        aux_names.append("c1")
    if add_bias2:
        aux_names.append("c2")
    if gb1:
        aux_names += ["g1", "b1"]
    if gb2:
        aux_names += ["g2", "b2"]
    aux = {n: nc.dram_tensor(n, (1, D), f32, kind="ExternalInput").ap()
           for n in aux_names}
    out1 = nc.dram_tensor("out1", (B_CORE, D), f32, kind="ExternalOutput").ap()
    out2 = nc.dram_tensor("out2", (B_CORE, D), f32, kind="ExternalOutput").ap()

    with tile.TileContext(nc) as tc:
        import contextlib
        with contextlib.ExitStack() as ctx:
            const = ctx.enter_context(tc.tile_pool(name="const", bufs=1))
            feat = ctx.enter_context(tc.tile_pool(name="feat", bufs=3))
            kvtp = ctx.enter_context(tc.tile_pool(name="kvtp", bufs=3))
            sp = ctx.enter_context(tc.tile_pool(name="sp", bufs=3))
            op = ctx.enter_context(tc.tile_pool(name="op", bufs=3))
            stat = ctx.enter_context(tc.tile_pool(name="stat", bufs=6))
            psum_t = ctx.enter_context(
                tc.tile_pool(name="psum_t", bufs=4, space="PSUM"))
            psum_o = ctx.enter_context(
                tc.tile_pool(name="psum_o", bufs=2, space="PSUM"))

            ident = const.tile([P, P], f32, tag="ident")
            make_identity(nc, ident)
            eps = const.tile([P, 1], f32, tag="eps")
            nc.vector.memset(eps, LN_EPS)

            # weights: direct f32r DMA, one [P, D] chunk per contraction slice
            # so the first matmuls only wait for their own chunk
            w_chunks = {}
            for mod, w_dram in ((1, w1t), (2, w2t)):
                w_chunks[mod] = []
                for j in range(KO):
                    wt = const.tile([P, D], f32r, tag=f"w{mod}_{j}")
                    nc.sync.dma_start(wt, w_dram[:, j, :])
                    w_chunks[mod].append(wt)

            # broadcast-replicated aux rows ([1, D] dram -> [P, D] sbuf)
            aux_sb = {}
            for n, ap in aux.items():
                t = const.tile([P, D], f32, tag=n)
                bcast = bass.AP(tensor=ap.tensor, offset=ap.offset,
                                ap=[[0, P], ap.ap[1]])
                nc.sync.dma_start(t, bcast)
                aux_sb[n] = t

            for rt in range(RT):
                rows = slice(rt * P, (rt + 1) * P)
                img_sb = feat.tile([P, D], f32, tag="img")
                nc.sync.dma_start(img_sb, img[rows, :])
                txt_sb = feat.tile([P, D], f32, tag="txt")
                nc.sync.dma_start(txt_sb, txt[rows, :])

                # modality 1: kv=txt, residual=img -> out1
                # modality 2: kv=img, residual=txt -> out2
                for mod, kv_sb, res_sb, out_d, biask, gbk in (
                    (1, txt_sb, img_sb, out1, add_bias1, gb1),
                    (2, img_sb, txt_sb, out2, add_bias2, gb2),
                ):
                    kvT = kvtp.tile([P, D], f32r, tag="kvT")
                    for half in range(2):
                        ps_t = psum_t.tile([P, 512], f32, tag="ps_t")
                        for jj in range(4):
                            j = half * 4 + jj
                            nc.tensor.transpose(
                                ps_t[:, jj * P:(jj + 1) * P],
                                kv_sb[:, j * P:(j + 1) * P],
                                ident)
                        nc.any.tensor_copy(
                            out=kvT[:, half * 512:(half + 1) * 512],
                            in_=ps_t)

                    s_sb = sp.tile([P, D], f32, tag="s")
                    ps = [psum_o.tile([P, 512], f32, tag=f"ps_o{nh}",
                                      name=f"ps_o{nh}")
                          for nh in range(2)]
                    # j-outer so matmul j only waits on weight chunk j
                    for j in range(KO):
                        for nh in range(2):
                            nc.tensor.matmul(
                                ps[nh],
                                kvT[:, j * P:(j + 1) * P],
                                w_chunks[mod][j][:, nh * 512:(nh + 1) * 512],
                                start=(j == 0), stop=(j == KO - 1))
                    for nh in range(2):
                        ncol = slice(nh * 512, (nh + 1) * 512)
                        # s = matmul + residual
                        nc.vector.tensor_add(
                            out=s_sb[:, ncol], in0=ps[nh], in1=res_sb[:, ncol])
                        if biask:
                            nc.vector.tensor_add(
                                out=s_sb[:, ncol], in0=s_sb[:, ncol],
                                in1=aux_sb[f"c{mod}"][:, ncol])

                    # layernorm over free axis
                    stats = stat.tile([P, 2, 6], f32, tag="stats")
                    nc.vector.bn_stats(stats[:, 0, :], s_sb[:, 0:512])
                    nc.vector.bn_stats(stats[:, 1, :], s_sb[:, 512:1024])
                    mv = stat.tile([P, 2], f32, tag="mv")
                    nc.vector.bn_aggr(mv, stats)
                    # mv[:,1] = 1/sqrt(var + eps)
                    nc.scalar.activation(
                        out=mv[:, 1:2], in_=mv[:, 1:2],
                        func=mybir.ActivationFunctionType.Sqrt,
                        bias=eps, scale=1.0)
                    nc.vector.reciprocal(mv[:, 1:2], mv[:, 1:2])
                    # nb = -mu * rstd, so ACT computes (s*rstd + nb) = (s-mu)*rstd
                    nb = stat.tile([P, 1], f32, tag="nb")
                    nc.vector.tensor_scalar(
                        out=nb, in0=mv[:, 0:1],
                        scalar1=mv[:, 1:2], scalar2=-1.0,
                        op0=mybir.AluOpType.mult,
                        op1=mybir.AluOpType.mult)

                    o_sb = op.tile([P, D], f32, tag="o")
                    nc.scalar.activation(
                        out=o_sb, in_=s_sb,
                        func=mybir.ActivationFunctionType.Identity,
                        bias=nb, scale=mv[:, 1:2])
                    if gbk:
                        nc.vector.tensor_mul(
                            out=o_sb, in0=o_sb, in1=aux_sb[f"g{mod}"])
                        nc.vector.tensor_add(
                            out=o_sb, in0=o_sb, in1=aux_sb[f"b{mod}"])
                    nc.sync.dma_start(out_d[rows, :], o_sb)

    nc.compile()
    return nc


def _fold(in_w, in_b, out_w, out_b):
    Dv = out_w.shape[0]
    Wv = in_w[2 * Dv:3 * Dv, :].astype(np.float64)
    bv = in_b[2 * Dv:3 * Dv].astype(np.float64)
    W = (out_w.astype(np.float64) @ Wv).astype(np.float32)
    c = (bv @ out_w.astype(np.float64).T + out_b.astype(np.float64)
         ).astype(np.float32)
    # rearrange W.T [k, n] -> [p, j, n] with k = j*128 + p
    wt = np.ascontiguousarray(
        W.T.reshape(KO, P, D).transpose(1, 0, 2)).astype(np.float32)
    return wt, c


def kernel(image_features, text_features,
           in_w1, in_b1, out_w1, out_b1,
           in_w2, in_b2, out_w2, out_b2,
           ln1_g, ln1_b, ln2_g, ln2_b):
    from concourse import bass_utils

    image_features = np.ascontiguousarray(image_features, dtype=np.float32)
    text_features = np.ascontiguousarray(text_features, dtype=np.float32)

    w1t, c1 = _fold(np.asarray(in_w1), np.asarray(in_b1),
                    np.asarray(out_w1), np.asarray(out_b1))
    w2t, c2 = _fold(np.asarray(in_w2), np.asarray(in_b2),
                    np.asarray(out_w2), np.asarray(out_b2))

    flags = (bool(np.any(c1)), bool(np.any(c2)),
             bool(np.any(np.asarray(ln1_g) != 1) or np.any(np.asarray(ln1_b))),
             bool(np.any(np.asarray(ln2_g) != 1) or np.any(np.asarray(ln2_b))))

    if flags not in _PROGRAM_CACHE:
        _PROGRAM_CACHE[flags] = _build_program(flags)
    nc = _PROGRAM_CACHE[flags]

    in_maps = []
    for c in range(N_CORES):
        rows = slice(c * B_CORE, (c + 1) * B_CORE)
        m = {
            "img": np.ascontiguousarray(image_features[rows]),
            "txt": np.ascontiguousarray(text_features[rows]),
            "w1t": w1t,
            "w2t": w2t,
        }
        if flags[0]:
            m["c1"] = c1.reshape(1, D)
        if flags[1]:
            m["c2"] = c2.reshape(1, D)
        if flags[2]:
            m["g1"] = np.asarray(ln1_g, np.float32).reshape(1, D)
            m["b1"] = np.asarray(ln1_b, np.float32).reshape(1, D)
        if flags[3]:
            m["g2"] = np.asarray(ln2_g, np.float32).reshape(1, D)
            m["b2"] = np.asarray(ln2_b, np.float32).reshape(1, D)
        in_maps.append(m)

    global _LAST_IN_MAPS
    _LAST_IN_MAPS = in_maps
    res = bass_utils.run_bass_kernel_spmd(nc, in_maps, list(range(N_CORES)))
    attended_image = np.concatenate(
        [res.results[c]["out1"] for c in range(N_CORES)], axis=0)
    attended_text = np.concatenate(
        [res.results[c]["out2"] for c in range(N_CORES)], axis=0)
    return attended_image, attended_text


# revision 17
# speedup vs baseline: 2.8629x; 1.0775x over previous
"""CrossModalAttention Trainium2 kernel.

Math: with seq_len=1 on both query and key/value sides, softmax over the
single key is exactly 1.0, so MHA(q_in, kv_in) == (kv_in @ Wv.T + bv) @ out_w.T + out_b.
Folding the two projections on the host (in float64):
    W = out_w @ Wv          c = bv @ out_w.T + out_b
gives   out_m = LayerNorm(kv @ W.T + c + residual) * g + b.

Device work per modality: one [B,1024]x[1024,1024] matmul + residual add +
LayerNorm.  Sharding: pure data parallel over the batch dim, 8 cores.
"""

import numpy as np

P = 128          # partitions
D = 1024         # hidden dim
KO = D // P      # 8 contraction chunks
N_CORES = 8
B_FULL = 16384
B_CORE = B_FULL // N_CORES   # 2048
RT = B_CORE // P             # 16 row tiles per core
LN_EPS = 1e-5

_PROGRAM_CACHE = {}


def _build_program(flags):
    """Build the Bass program. flags = (add_bias1, add_bias2, gb1, gb2)."""
    import concourse.bass as bass
    import concourse.bacc as bacc
    import concourse.tile as tile
    from concourse import mybir
    from concourse.masks import make_identity
    from concourse._compat import get_trn_type

    add_bias1, add_bias2, gb1, gb2 = flags
    f32 = mybir.dt.float32
    f32r = mybir.dt.float32r

    nc = bacc.Bacc(get_trn_type() or "TRN2", target_bir_lowering=False,
                   debug=False, num_devices=N_CORES)

    img = nc.dram_tensor("img", (B_CORE, D), f32, kind="ExternalInput").ap()
    txt = nc.dram_tensor("txt", (B_CORE, D), f32, kind="ExternalInput").ap()
    # weights pre-arranged on host: w[p, j, n] = W[n, j*128+p]  (i.e. W.T chunked)
    w1t = nc.dram_tensor("w1t", (P, KO, D), f32r, kind="ExternalInput").ap()
    w2t = nc.dram_tensor("w2t", (P, KO, D), f32r, kind="ExternalInput").ap()
    aux_names = []
    if add_bias1:
        aux_names.append("c1")
    if add_bias2:
        aux_names.append("c2")
    if gb1:
        aux_names += ["g1", "b1"]
    if gb2:
        aux_names += ["g2", "b2"]
    aux = {n: nc.dram_tensor(n, (1, D), f32, kind="ExternalInput").ap()
           for n in aux_names}
    out1 = nc.dram_tensor("out1", (B_CORE, D), f32, kind="ExternalOutput").ap()
    out2 = nc.dram_tensor("out2", (B_CORE, D), f32, kind="ExternalOutput").ap()

    with tile.TileContext(nc) as tc:
        import contextlib
        with contextlib.ExitStack() as ctx:
            const = ctx.enter_context(tc.tile_pool(name="const", bufs=1))
            feat = ctx.enter_context(tc.tile_pool(name="feat", bufs=4))
            kvtp = ctx.enter_context(tc.tile_pool(name="kvtp", bufs=3))
            sp = ctx.enter_context(tc.tile_pool(name="sp", bufs=3))
            op = ctx.enter_context(tc.tile_pool(name="op", bufs=3))
            stat = ctx.enter_context(tc.tile_pool(name="stat", bufs=6))
            psum_t = ctx.enter_context(
                tc.tile_pool(name="psum_t", bufs=4, space="PSUM"))
            psum_o = ctx.enter_context(
                tc.tile_pool(name="psum_o", bufs=2, space="PSUM"))

            ident = const.tile([P, P], f32, tag="ident")
            make_identity(nc, ident)
            eps = const.tile([P, 1], f32, tag="eps")
            nc.vector.memset(eps, LN_EPS)

            # prefetch the first row tiles' features BEFORE the 8MB of
            # weights so the PE transpose pipeline starts immediately
            prefetched = {}
            for rt in range(2):
                rows = slice(rt * P, (rt + 1) * P)
                pimg = feat.tile([P, D], f32, tag="img", name=f"pimg{rt}")
                nc.sync.dma_start(pimg, img[rows, :])
                ptxt = feat.tile([P, D], f32, tag="txt", name=f"ptxt{rt}")
                nc.sync.dma_start(ptxt, txt[rows, :])
                prefetched[rt] = (pimg, ptxt)

            # weights: direct f32r DMA, one [P, D] chunk per contraction slice
            # so the first matmuls only wait for their own chunk
            w_chunks = {}
            for mod, w_dram in ((1, w1t), (2, w2t)):
                w_chunks[mod] = []
                for j in range(KO):
                    wt = const.tile([P, D], f32r, tag=f"w{mod}_{j}")
                    nc.sync.dma_start(wt, w_dram[:, j, :])
                    w_chunks[mod].append(wt)

            # broadcast-replicated aux rows ([1, D] dram -> [P, D] sbuf)
            aux_sb = {}
            for n, ap in aux.items():
                t = const.tile([P, D], f32, tag=n)
                bcast = bass.AP(tensor=ap.tensor, offset=ap.offset,
                                ap=[[0, P], ap.ap[1]])
                nc.sync.dma_start(t, bcast)
                aux_sb[n] = t

            for rt in range(RT):
                rows = slice(rt * P, (rt + 1) * P)
                if rt in prefetched:
                    img_sb, txt_sb = prefetched[rt]
                else:
                    img_sb = feat.tile([P, D], f32, tag="img")
                    nc.sync.dma_start(img_sb, img[rows, :])
                    txt_sb = feat.tile([P, D], f32, tag="txt")
                    nc.sync.dma_start(txt_sb, txt[rows, :])

                # modality 1: kv=txt, residual=img -> out1
                # modality 2: kv=img, residual=txt -> out2
                for mod, kv_sb, res_sb, out_d, biask, gbk in (
                    (1, txt_sb, img_sb, out1, add_bias1, gb1),
                    (2, img_sb, txt_sb, out2, add_bias2, gb2),
                ):
                    kvT = kvtp.tile([P, D], f32r, tag="kvT")
                    for half in range(2):
                        ps_t = psum_t.tile([P, 512], f32, tag="ps_t")
                        for jj in range(4):
                            j = half * 4 + jj
                            nc.tensor.transpose(
                                ps_t[:, jj * P:(jj + 1) * P],
                                kv_sb[:, j * P:(j + 1) * P],
                                ident)
                        nc.any.tensor_copy(
                            out=kvT[:, half * 512:(half + 1) * 512],
                            in_=ps_t)

                    s_sb = sp.tile([P, D], f32, tag="s")
                    ps = [psum_o.tile([P, 512], f32, tag=f"ps_o{nh}",
                                      name=f"ps_o{nh}")
                          for nh in range(2)]
                    # j-outer so matmul j only waits on weight chunk j
                    for j in range(KO):
                        for nh in range(2):
                            nc.tensor.matmul(
                                ps[nh],
                                kvT[:, j * P:(j + 1) * P],
                                w_chunks[mod][j][:, nh * 512:(nh + 1) * 512],
                                start=(j == 0), stop=(j == KO - 1))
                    for nh in range(2):
                        ncol = slice(nh * 512, (nh + 1) * 512)
                        # s = matmul + residual
                        nc.vector.tensor_add(
                            out=s_sb[:, ncol], in0=ps[nh], in1=res_sb[:, ncol])
                        if biask:
                            nc.vector.tensor_add(
                                out=s_sb[:, ncol], in0=s_sb[:, ncol],
                                in1=aux_sb[f"c{mod}"][:, ncol])

                    # layernorm over free axis
                    stats = stat.tile([P, 2, 6], f32, tag="stats")
                    nc.vector.bn_stats(stats[:, 0, :], s_sb[:, 0:512])
                    nc.vector.bn_stats(stats[:, 1, :], s_sb[:, 512:1024])
                    mv = stat.tile([P, 2], f32, tag="mv")
                    nc.vector.bn_aggr(mv, stats)
                    # mv[:,1] = 1/sqrt(var + eps)
                    nc.scalar.activation(
                        out=mv[:, 1:2], in_=mv[:, 1:2],
                        func=mybir.ActivationFunctionType.Sqrt,
                        bias=eps, scale=1.0)
                    nc.vector.reciprocal(mv[:, 1:2], mv[:, 1:2])
                    # nb = -mu * rstd, so ACT computes (s*rstd + nb) = (s-mu)*rstd
                    nb = stat.tile([P, 1], f32, tag="nb")
                    nc.vector.tensor_scalar(
                        out=nb, in0=mv[:, 0:1],
                        scalar1=mv[:, 1:2], scalar2=-1.0,
                        op0=mybir.AluOpType.mult,
                        op1=mybir.AluOpType.mult)

                    o_sb = op.tile([P, D], f32, tag="o")
                    nc.scalar.activation(
                        out=o_sb, in_=s_sb,
                        func=mybir.ActivationFunctionType.Identity,
                        bias=nb, scale=mv[:, 1:2])
                    if gbk:
                        nc.vector.tensor_mul(
                            out=o_sb, in0=o_sb, in1=aux_sb[f"g{mod}"])
                        nc.vector.tensor_add(
                            out=o_sb, in0=o_sb, in1=aux_sb[f"b{mod}"])
                    nc.sync.dma_start(out_d[rows, :], o_sb)

    nc.compile()
    return nc


def _fold(in_w, in_b, out_w, out_b):
    Dv = out_w.shape[0]
    Wv = in_w[2 * Dv:3 * Dv, :].astype(np.float64)
    bv = in_b[2 * Dv:3 * Dv].astype(np.float64)
    W = (out_w.astype(np.float64) @ Wv).astype(np.float32)
    c = (bv @ out_w.astype(np.float64).T + out_b.astype(np.float64)
         ).astype(np.float32)
    # rearrange W.T [k, n] -> [p, j, n] with k = j*128 + p
    wt = np.ascontiguousarray(
        W.T.reshape(KO, P, D).transpose(1, 0, 2)).astype(np.float32)
    return wt, c


def kernel(image_features, text_features,
           in_w1, in_b1, out_w1, out_b1,
           in_w2, in_b2, out_w2, out_b2,
           ln1_g, ln1_b, ln2_g, ln2_b):
    from concourse import bass_utils

    image_features = np.ascontiguousarray(image_features, dtype=np.float32)
    text_features = np.ascontiguousarray(text_features, dtype=np.float32)

    w1t, c1 = _fold(np.asarray(in_w1), np.asarray(in_b1),
                    np.asarray(out_w1), np.asarray(out_b1))
    w2t, c2 = _fold(np.asarray(in_w2), np.asarray(in_b2),
                    np.asarray(out_w2), np.asarray(out_b2))

    flags = (bool(np.any(c1)), bool(np.any(c2)),
             bool(np.any(np.asarray(ln1_g) != 1) or np.any(np.asarray(ln1_b))),
             bool(np.any(np.asarray(ln2_g) != 1) or np.any(np.asarray(ln2_b))))

    if flags not in _PROGRAM_CACHE:
        _PROGRAM_CACHE[flags] = _build_program(flags)
    nc = _PROGRAM_CACHE[flags]

    in_maps = []
    for c in range(N_CORES):
        rows = slice(c * B_CORE, (c + 1) * B_CORE)
        m = {
            "img": np.ascontiguousarray(image_features[rows]),
            "txt": np.ascontiguousarray(text_features[rows]),
            "w1t": w1t,
            "w2t": w2t,
        }
        if flags[0]:
            m["c1"] = c1.reshape(1, D)
        if flags[1]:
            m["c2"] = c2.reshape(1, D)
        if flags[2]:
            m["g1"] = np.asarray(ln1_g, np.float32).reshape(1, D)
            m["b1"] = np.asarray(ln1_b, np.float32).reshape(1, D)
        if flags[3]:
            m["g2"] = np.asarray(ln2_g, np.float32).reshape(1, D)
            m["b2"] = np.asarray(ln2_b, np.float32).reshape(1, D)
        in_maps.append(m)

    global _LAST_IN_MAPS
    _LAST_IN_MAPS = in_maps
    res = bass_utils.run_bass_kernel_spmd(nc, in_maps, list(range(N_CORES)))
    attended_image = np.concatenate(
        [res.results[c]["out1"] for c in range(N_CORES)], axis=0)
    attended_text = np.concatenate(
        [res.results[c]["out2"] for c in range(N_CORES)], axis=0)
    return attended_image, attended_text
